# revision 12
# baseline (speedup 1.0000x reference)
"""DCGRU cell Trainium2 kernel (8-core data-parallel over batch).

Math (per core, B_loc=4):
  gconv(x, W, b) = sum_m (A_m x) @ W_m + b,  A = [I, S0, 2S0^2-I, S1, 2S1^2-I]
  value = sigmoid(gconv1(concat(inp, hx)));  r, u = split(value)
  c = tanh(gconv2(concat(inp, r*hx)));  new = u*hx + (1-u)*c

Device layout:
  Diffusion runs as out[n_tile,(b,c)] += ST[m-pair, n-tile]^T @ x[m-pair,(b,c)]
  in fp8(e4m3) DoubleRow perf mode (2 K-tiles per matmul, 2x PE throughput),
  fp32 PSUM.  Supports are host-scaled by SS=2048 into fp8 range; the inverse
  scale is folded into the PSUM-evacuating vector ops.  Both supports are
  SBUF-resident in fp8 (32KB/partition each).  Hop-1 results are evacuated
  twice: fp16 (dense-stage input, stored as 2*y so the Chebyshev combine is
  one op; W rows for m in {1,3} halved on host) and fp8 scaled by SX1 (rhs of
  the hop-2 S-apply, evacuated on the scalar engine).  Dense stage consumes
  channel-major xT tiles built by PE transposes; the bias is a ones-row
  appended to x0T.  gconv2 reuses the input-channel diffusion from gconv1
  (those channels don't change) and overwrites only u-columns.
"""

import sys

if "/opt/trn_rl_repo" not in sys.path:
    sys.path.insert(0, "/opt/trn_rl_repo")

import ml_dtypes
import numpy as np

import concourse.bass as bass
import concourse.mybir as mybir
import concourse.tile as tile
from concourse.bass_utils import run_bass_kernel_spmd
from concourse.masks import make_identity

F8 = mybir.dt.float8e4
F16 = mybir.dt.float16
F32 = mybir.dt.float32
DR = mybir.MatmulPerfMode.DoubleRow

N = 2048          # nodes
U = 64            # units
D = 2             # input dim
C = D + U         # 66 channels after concat
M = 5             # diffusion matrices
B = 32            # global batch
NCORES = 8
BL = B // NCORES  # 4 per-core batch
NT = N // 128     # 16 node tiles
NQ = 16           # dense-stage chunks

SS = 2048.0       # support fp8 scale (entries in [0,1) after scaling)
SX1 = 64.0        # hop-1 fp8 copy scale (y1 entries ~0.015 std -> ~1 std)


def _split_drain_waits(nc):
    """This walrus build accepts only one sync-wait per instruction on several
    ISA formats; hoist extra waits onto single-wait NoOps placed before (same
    engine, so program order preserves the semantics)."""
    cnt = 0
    for f in nc.m.functions:
        for blk in f.blocks:
            new = []
            for inst in blk.instructions:
                si = inst.sync_info
                if si is not None and len(si.on_wait) > 1:
                    waits = list(si.on_wait)
                    for w in waits[:-1]:
                        cnt += 1
                        n = mybir.InstNoOp(name=f"I-dsplit-{cnt}", ins=[], outs=[])
                        n.engine = inst.engine
                        n.sync_info = mybir.SyncInfo(on_wait=[w], on_update=[])
                        new.append(n)
                    inst.sync_info = mybir.SyncInfo(
                        on_wait=[waits[-1]], on_update=list(si.on_update)
                    )
                new.append(inst)
            blk.instructions = new
    return cnt


def _build_nc():
    nc = bass.Bass()

    # DRAM parameters (host-prepped layouts)
    d_st0 = nc.dram_tensor("st0", [NT, 128, N], F8, kind="ExternalInput")
    d_st1 = nc.dram_tensor("st1", [NT, 128, NT, 128], F8, kind="ExternalInput")
    d_x0 = nc.dram_tensor("x0h", [NT, 128, BL, C], F16, kind="ExternalInput")
    d_x08 = nc.dram_tensor("x08", [NT, 128, BL, C], F8, kind="ExternalInput")
    d_x0t = nc.dram_tensor("x0t", [C + 1, BL, N], F16, kind="ExternalInput")
    d_hxf = nc.dram_tensor("hxf", [NT, 128, BL, U], F16, kind="ExternalInput")
    d_wru = nc.dram_tensor("wru", [C + 1, M, 2 * U], F16, kind="ExternalInput")
    d_wc = nc.dram_tensor("wc", [C + 1, M, U], F16, kind="ExternalInput")
    d_out = nc.dram_tensor("out", [NT, 128, BL, U], F32, kind="ExternalOutput")

    with tile.TileContext(nc) as tc:
        with (
            tc.tile_pool(name="const", bufs=1) as const_pool,
            tc.tile_pool(name="xbufs", bufs=1) as xbufs,
            tc.tile_pool(name="xtq", bufs=3) as xtq_pool,
            tc.tile_pool(name="cbuf", bufs=2) as c_pool,
            tc.tile_pool(name="dps", bufs=3, space="PSUM") as diff_ps,
            tc.tile_pool(name="tps", bufs=3, space="PSUM") as tr_ps,
            tc.tile_pool(name="nps", bufs=2, space="PSUM") as dense_ps,
        ):
            # ---- resident constants (chunked so compute starts early) ----
            # st0 is on the critical path (first matmul group needs all of
            # it); split it across the sync and gpsimd DMA queues so both
            # queues pull concurrently.  st1 follows (needed ~17us later).
            x08 = xbufs.tile([128, NT, BL, C], F8, tag="x08")
            for g in range(4):
                eng = nc.sync if g % 2 == 0 else nc.gpsimd
                eng.dma_start(
                    out=x08[:, 4 * g:4 * g + 4, :, :],
                    in_=d_x08[4 * g:4 * g + 4].rearrange("t p b c -> p t b c"),
                )
            st0 = const_pool.tile([128, NT, N], F8)
            for mc in range(NT):
                eng = nc.sync if mc % 2 == 0 else nc.gpsimd
                eng.dma_start(out=st0[:, mc, :], in_=d_st0[mc])
            st1 = const_pool.tile([128, NT, NT, 128], F8)
            for nt in range(NT):
                eng = nc.sync if nt % 2 == 0 else nc.gpsimd
                eng.dma_start(out=st1[:, nt], in_=d_st1[nt])
            # x0/hxf go on the gpsimd queue so the (slow, strided) loads don't
            # delay the sync queue's per-chunk x0t loads needed by dense-0
            x0 = xbufs.tile([128, NT, BL, C], F16, tag="x0")
            for g in range(4):
                nc.gpsimd.dma_start(
                    out=x0[:, 4 * g:4 * g + 4, :, :],
                    in_=d_x0[4 * g:4 * g + 4].rearrange("t p b c -> p t b c"),
                )
            ident = const_pool.tile([128, 128], F16)
            make_identity(nc, ident)
            wall = const_pool.tile([C + 1, M, 3 * U], F16)
            wru = wall[:, :, 0:2 * U]
            wc = wall[:, :, 2 * U:3 * U]
            nc.sync.dma_start(out=wru, in_=d_wru[:, :, :])
            nc.sync.dma_start(out=wc, in_=d_wc[:, :, :])
            hxf = xbufs.tile([128, NT, BL, U], F16, tag="hxf")
            nc.gpsimd.dma_start(out=hxf, in_=d_hxf[:].rearrange("t p b u -> p t (b u)"))

            # diffusion outputs (m=1..4), full 66 channels, fp16
            xh = [xbufs.tile([128, NT, BL, C], F16, tag=f"xh{i}", name=f"xh{i}")
                  for i in range(4)]
            # fp8 copies of hop-1 results (rhs of hop-2), one per support
            x18 = [xbufs.tile([128, NT, BL, C], F8, tag=f"x18_{s}", name=f"x18_{s}")
                   for s in range(2)]
            x18g1 = [xbufs.tile([128, NT, BL, U], F8, tag=f"x18g1_{s}",
                                name=f"x18g1_{s}") for s in range(2)]
            # gconv2 state r*hx (u-columns only)
            xhp = xbufs.tile([128, NT, BL, U], F16, tag="xhp")
            xhp8 = xbufs.tile([128, NT, BL, U], F8, tag="xhp8")
            r_sb = xbufs.tile([128, NT, BL, U], F16, tag="r")
            u_sb = xbufs.tile([128, NT, BL, U], F16, tag="u")

            def diffusion(gi):
                """4 S-applications; writes xh[0..3] (u-cols only when gi=1)."""
                nfree = C if gi == 0 else U
                x_first8 = x08 if gi == 0 else xhp8
                for s, hop in [(0, 0), (1, 0), (0, 1), (1, 1)]:
                    dst = xh[2 * s + hop]
                    d8buf = x18[s] if gi == 0 else x18g1[s]
                    for nt in range(NT):
                        ps = diff_ps.tile([128, BL, nfree], F32, tag="dps")
                        for mc in range(0, NT, 2):
                            if s == 0:
                                lhsT = st0[:, mc:mc + 2, nt * 128:(nt + 1) * 128]
                            else:
                                lhsT = st1[:, nt, mc:mc + 2, :]
                            if hop == 0:
                                rhs = x_first8[:, mc:mc + 2]
                            else:
                                rhs = d8buf[:, mc:mc + 2]
                            nc.tensor.matmul(
                                ps, lhsT, rhs,
                                start=(mc == 0), stop=(mc == NT - 2),
                                perf_mode=DR,
                            )
                        if gi == 0:
                            dst_ap = dst[:, nt, :, :]
                            sub = x0[:, nt, :, :]
                        else:
                            dst_ap = dst[:, nt, :, 0:U]
                            sub = xhp[:, nt, :, :]
                        if hop == 0:
                            # xh = 2*y1 (stored scaled; W rows halved on host)
                            nc.vector.tensor_scalar_mul(dst_ap, ps, 2.0 / SS)
                            # fp8 copy = SX1*y1 for the hop-2 S-apply
                            nc.scalar.activation(
                                out=d8buf[:, nt],
                                in_=ps,
                                func=mybir.ActivationFunctionType.Copy,
                                scale=SX1 / SS,
                            )
                        else:
                            # x2 = 2*(S y1) - x0
                            nc.vector.scalar_tensor_tensor(
                                out=dst_ap,
                                in0=ps,
                                scalar=2.0 / (SS * SX1),
                                in1=sub,
                                op0=mybir.AluOpType.mult,
                                op1=mybir.AluOpType.subtract,
                            )

            def stage_transposes(gi, q):
                """PE transposes + DVE copies building this chunk's xT tiles."""
                nt = q
                qs = slice(nt * 128, (nt + 1) * 128)
                xtc = xtq_pool.tile([C + 1, M, BL, 128], F16, tag="xt",
                                    name=f"xt_g{gi}_q{q}")
                if gi == 0:
                    nc.sync.dma_start(out=xtc[:, 0, :, :], in_=d_x0t[:, :, qs])
                else:
                    nc.sync.dma_start(
                        out=xtc[U:C + 1, 0, :, :], in_=d_x0t[U:C + 1, :, qs]
                    )
                    pst = tr_ps.tile([C, BL, 128], F16, tag="tps", name="pst0")
                    for b in range(BL):
                        nc.tensor.transpose(pst[0:U, b, :], xhp[:, nt, b, :], ident)
                    nc.vector.tensor_copy(xtc[0:U, 0, :, :], pst[0:U])
                for m in range(1, M):
                    srcb = xh[m - 1]
                    pst = tr_ps.tile([C, BL, 128], F16, tag="tps", name="pstm")
                    for b in range(BL):
                        nc.tensor.transpose(pst[:, b, :], srcb[:, nt, b, :], ident)
                    # all PSUM->SBUF transpose evacs on DVE; ScalarE keeps the
                    # activations, GpSimd the SBUF-only combines.  Per-chunk
                    # DVE load (~2us) stays under the PE's ~2.5us of matmuls.
                    nc.vector.tensor_copy(xtc[0:C, m, :, :], pst)
                return xtc

            def stage_dense(gi, q, xtc):
                w_sb = wru if gi == 0 else wc
                osz = 2 * U if gi == 0 else U
                nt = q
                dps = dense_ps.tile([128, BL, osz], F32, tag="nps")
                for b in range(BL):
                    for m in range(M):
                        k = C + 1 if m == 0 else C
                        nc.tensor.matmul(
                            dps[:, b, :],
                            xtc[0:k, m, b, :],
                            w_sb[0:k, m, :],
                            start=(m == 0),
                            stop=(m == M - 1),
                        )
                if gi == 0:
                    nc.scalar.activation(
                        out=r_sb[:, nt, :, :],
                        in_=dps[:, :, 0:U],
                        func=mybir.ActivationFunctionType.Sigmoid,
                    )
                    nc.scalar.activation(
                        out=u_sb[:, nt, :, :],
                        in_=dps[:, :, U:2 * U],
                        func=mybir.ActivationFunctionType.Sigmoid,
                    )
                    nc.vector.tensor_mul(
                        xhp[:, nt, :, :], r_sb[:, nt, :, :], x0[:, nt, :, 0:U]
                    )
                    nc.scalar.activation(
                        out=xhp8[:, nt],
                        in_=xhp[:, nt],
                        func=mybir.ActivationFunctionType.Copy,
                        scale=1.0,
                    )
                else:
                    cpair = c_pool.tile([128, 2, BL, U], F32, tag="cb")
                    cb = cpair[:, 0]
                    tmp = cpair[:, 1]
                    nc.scalar.activation(
                        out=cb, in_=dps, func=mybir.ActivationFunctionType.Tanh
                    )
                    # new = c + u*(hx - c); SBUF-only chain runs on GpSimd to
                    # keep DVE free for the transpose-evac copies
                    nc.gpsimd.tensor_sub(tmp, hxf[:, nt, :, :], cb)
                    nc.gpsimd.tensor_mul(tmp, u_sb[:, nt, :, :], tmp)
                    nc.gpsimd.tensor_add(tmp, tmp, cb)
                    nc.sync.dma_start(out=d_out[nt], in_=tmp)

            def dense_quarters(gi):
                # software pipeline: transposes for chunk q+1 issue on PE before
                # dense of chunk q, hiding the DVE copy latency and keeping
                # real matmuls interleaved with transpose-mode ops (HAM warmth)
                prev = stage_transposes(gi, 0)
                for q in range(1, NQ):
                    cur = stage_transposes(gi, q)
                    stage_dense(gi, q - 1, prev)
                    prev = cur
                stage_dense(gi, NQ - 1, prev)

            diffusion(0)
            dense_quarters(0)
            diffusion(1)
            dense_quarters(1)

    _split_drain_waits(nc)
    return nc


_NC_CACHE = None


def _get_nc():
    global _NC_CACHE
    if _NC_CACHE is None:
        _NC_CACHE = _build_nc()
    return _NC_CACHE


def _prep_host(inputs, hx, support0, support1, W_ru, b_ru, W_c, b_c):
    f16 = np.float16
    f8 = ml_dtypes.float8_e4m3
    inp = inputs.reshape(B, N, D).astype(np.float32)
    hx3 = hx.reshape(B, N, U).astype(np.float32)
    x0_full = np.concatenate([hx3, inp], axis=2)  # [B, N, C] fp32, u-first

    st0 = (np.ascontiguousarray(support0.T) * SS).astype(f8).reshape(NT, 128, N)
    st1 = (
        np.ascontiguousarray(
            support1.T.reshape(NT, 128, NT, 128).transpose(2, 1, 0, 3)
        ) * SS
    ).astype(f8)

    def prep_w(W, bvec, osz):
        w = W.reshape(C, M, osz).astype(np.float32)
        w = np.concatenate([w[D:], w[:D]], axis=0).copy()  # u-first rows
        w[:, 1, :] *= 0.5
        w[:, 3, :] *= 0.5
        wf = np.zeros((C + 1, M, osz), np.float32)
        wf[:C] = w
        wf[C, 0, :] = bvec
        return wf.astype(f16)

    wru = prep_w(W_ru, b_ru, 2 * U)
    wcc = prep_w(W_c, b_c, U)

    in_maps = []
    for c in range(NCORES):
        cs = slice(c * BL, (c + 1) * BL)
        x0c = x0_full[cs]                                   # [BL, N, C]
        x0h32 = np.ascontiguousarray(
            x0c.transpose(1, 0, 2).reshape(NT, 128, BL, C)
        )
        x0t = np.concatenate(
            [x0c.transpose(2, 0, 1), np.ones((1, BL, N), np.float32)], axis=0
        ).astype(f16)                                        # [C+1, BL, N]
        hxf = np.ascontiguousarray(
            hx3[cs].transpose(1, 0, 2).reshape(NT, 128, BL, U)
        ).astype(f16)
        in_maps.append(
            {
                "st0": st0,
                "st1": st1,
                "x0h": x0h32.astype(f16),
                "x08": x0h32.astype(f8),
                "x0t": np.ascontiguousarray(x0t),
                "hxf": hxf,
                "wru": wru,
                "wc": wcc,
            }
        )
    return in_maps


def kernel(inputs, hx, support0, support1, W_ru, b_ru, W_c, b_c, _trace=False,
           _tmpdir=None):
    nc = _get_nc()
    in_maps = _prep_host(
        inputs, hx, support0, support1, W_ru, b_ru, W_c, b_c
    )
    res = run_bass_kernel_spmd(
        nc, in_maps, core_ids=list(range(NCORES)), trace=_trace, tmpdir=_tmpdir
    )
    out = np.empty((B, N * U), np.float32)
    for c in range(NCORES):
        od = res.results[c]["out"]  # [NT, 128, BL, U]
        out[c * BL:(c + 1) * BL] = (
            od.transpose(2, 0, 1, 3).reshape(BL, N * U)
        )
    kernel._last_result = res
    return out
